# revision 14
# baseline (speedup 1.0000x reference)
"""NetVLAD forward on 8 Trainium2 NeuronCores — v2.

Host prep (inside kernel(), CPU): L2-normalize x over channels, cast f16,
and lay out two views: xn (channel-major, mm1 stationary) and rhs_pack
(position-major: [xnT_s0 | xnT_s1 | 1 1] per 128-position chunk, mm2
moving operand). Weights: w^T f16; b tiled 8x; centroids f32.

Device per core (2 samples, C=128, N=4096, K=64):
  mm1 (flipped): per 128-pos chunk, lhsT=xn_chunk -> l0t [n,K] slice in a
    PSUM bank (8 chunks per bank); then one rank-1 matmul per bank
    (ones[1,128] (x) b_tiled[1,512]) accumulates +b[k] onto the whole bank.
  exp: ACT per bank [128,512] PSUM->SBUF f16 (host-normed, b in PSUM).
  ssum = sum_k e0 (DVE reduce), rs = 1/ssum (DVE recip), es3 = e0*rs (DVE).
  mm2: per chunk, lhsT=es3 (both samples packed), rhs=rhs_pack chunk
    [xnT0|xnT1|1 1] -> vlad + A accumulate in PSUM. Bursts are emitted
    lagging two groups behind mm1 so the in-order PE queue never stalls.
  finalize: t2 = vlad - cent*A; row L2 norm; /sqrt(K); DMA out.
"""

import contextlib

import numpy as np

import concourse.bass as bass
import concourse.bacc as bacc
import concourse.tile as tile
from concourse import mybir
from concourse.bass_utils import run_bass_kernel_spmd

f32 = mybir.dt.float32
f16 = mybir.dt.float16
f8 = mybir.dt.float8e4
AF = mybir.ActivationFunctionType
ALU = mybir.AluOpType
AX = mybir.AxisListType

B, C, N, K = 16, 128, 4096, 64
NCORES = 8
BS = B // NCORES          # 2 samples per core
NCH = N // 128            # 32 chunks per sample
GCH = 8                   # chunks per PSUM bank / exp group
NG = NCH // GCH           # 4 bank-groups per sample
QW = GCH * 128            # 1024 columns of xn per group


def _bc(ap, ncols, rep):
    """[P, ncols] tile viewed as [P, ncols, rep], inner dim broadcast."""
    return bass.AP(tensor=ap.tensor, offset=ap.offset,
                   ap=[list(ap.ap[0]), [1, ncols], [0, rep]])


def _bcK(ap2d, rep):
    """[P, n] AP (any stride) viewed as [P, n, rep], inner broadcast."""
    return bass.AP(tensor=ap2d.tensor, offset=ap2d.offset,
                   ap=[list(ap2d.ap[0]), list(ap2d.ap[1]), [0, rep]])


def _build():
    nc = bacc.Bacc("TRN2", target_bir_lowering=False, debug=False,
                   num_devices=NCORES)
    xn_h = nc.dram_tensor("xn", [BS, NG, C, QW], f8, kind="ExternalInput")
    rhs_h = nc.dram_tensor("rhs", [128, NCH, 257], f8, kind="ExternalInput")
    w_h = nc.dram_tensor("wt", [C, K], f8, kind="ExternalInput")
    bp_h = nc.dram_tensor("bpack", [1, GCH * K], f16, kind="ExternalInput")
    c_h = nc.dram_tensor("cent", [K, C], f32, kind="ExternalInput")
    o_h = nc.dram_tensor("out", [BS, K * C], f32, kind="ExternalOutput")

    with tile.TileContext(nc) as tc:
        _emit(nc, tc, xn_h, rhs_h, w_h, bp_h, c_h, o_h)
    nc.compile()
    return nc


def _emit(nc, tc, xn_h, rhs_h, w_h, bp_h, c_h, o_h):
    ctx = contextlib.ExitStack()
    with ctx:
        const = ctx.enter_context(tc.tile_pool(name="const", bufs=1))
        fin = ctx.enter_context(tc.tile_pool(name="fin", bufs=4))
        ps_l = ctx.enter_context(tc.tile_pool(name="ps_l", bufs=6, space="PSUM"))
        ps_v = ctx.enter_context(tc.tile_pool(name="ps_v", bufs=1, space="PSUM"))

        # ---- constants (DMAs posted after xn q0 below) ----
        w_sb = const.tile([C, K], f8, tag="w_sb")
        bp_sb = const.tile([1, GCH * K], f16, tag="bp_sb")
        cent2 = const.tile([128, C], f32, tag="cent2")
        ones_row = const.tile([1, 128], f16, tag="ones_row")
        nc.vector.memset(ones_row[:], 1.0)
        # tiny early exp to pull the ACT Exp table load into the DMA window
        dmy = const.tile([1, 1], f32, tag="dmy")
        nc.vector.memset(dmy[:], 0.0)
        nc.scalar.activation(out=dmy[:], in_=dmy[:], func=AF.Exp)

        # ---- streamed inputs, quarter-granular and interleaved ----
        xn_sb = [const.tile([C, N], f8, tag=f"xn{s}", name=f"xn_sb{s}")
                 for s in range(BS)]
        rhs_sb = const.tile([128, NCH, 257], f8, tag="rhs")
        # first 8 posts = the 8 xn quarters -> unique HWDGE sem lanes;
        # later posts recycle lanes of earlier-landing transfers, which only
        # ever makes a waiter wait on something that finished earlier.
        def xn_span(s, q0, nq, eng):
            cs = slice(q0 * QW, (q0 + nq) * QW)
            src_ap = xn_h[s, q0:q0 + nq, :, :]        # [nq, C, QW]
            src_cqw = bass.AP(tensor=src_ap.tensor, offset=src_ap.offset,
                              ap=[list(src_ap.ap[1]), list(src_ap.ap[0]),
                                  list(src_ap.ap[2])])
            eng.dma_start(
                out=xn_sb[s][:, cs].rearrange("c (q w) -> c q w", q=nq),
                in_=src_cqw)
        # tiny first posts unblock pair 0 ASAP; one big post carries the rest
        xn_span(0, 0, 1, nc.sync)
        xn_span(1, 0, 1, nc.scalar)
        nc.gpsimd.dma_start(out=bp_sb[:], in_=bp_h[:, :])
        xn_span(0, 1, 3, nc.sync)
        xn_span(1, 1, 3, nc.scalar)
        nc.gpsimd.dma_start(out=w_sb[:], in_=w_h[:, :])
        nc.gpsimd.dma_start(out=rhs_sb[:, 0:2 * GCH, :],
                            in_=rhs_h[:, 0:2 * GCH, :])
        nc.gpsimd.dma_start(out=rhs_sb[:, 2 * GCH:NCH, :],
                            in_=rhs_h[:, 2 * GCH:NCH, :])
        nc.sync.dma_start(out=cent2[0:K, :], in_=c_h[:, :])
        nc.scalar.dma_start(out=cent2[K:128, :], in_=c_h[:, :])

        # pipeline tiles [128, ci, s, K] (chunk-major: mm2 lhsT slice is
        # contiguous) and per-position softmax scalars
        e0 = const.tile([128, NCH, BS, K], f16, tag="e0")
        es3 = const.tile([128, NCH, BS, K], f8, tag="es3")
        ssum = const.tile([128, BS, NCH], f32, tag="ssum")
        rs = const.tile([128, BS, NCH], f32, tag="rs")

        ps_vlad = ps_v.tile([128, 257], f32, tag="vlad")

        preseed = {}
        for j in range(3):
            for s in range(BS):
                psL = ps_l.tile([128, GCH * K], f32, tag="l0t",
                                name=f"psL_{j}_{s}")
                nc.tensor.matmul(psL[:], ones_row[0:1, :], bp_sb[0:1, :],
                                 start=True, stop=False)
                preseed[(j, s)] = psL

        def emit_pair(j):
            sl = slice(j * GCH, (j + 1) * GCH)
            psLs = []
            for s in range(BS):
                if (j, s) in preseed:
                    psLs.append(preseed[(j, s)])
                    continue
                psL = ps_l.tile([128, GCH * K], f32, tag="l0t",
                                name=f"psL_{j}_{s}")
                nc.tensor.matmul(psL[:], ones_row[0:1, :], bp_sb[0:1, :],
                                 start=True, stop=False)
                psLs.append(psL)
            for s in range(BS):
                for cc in range(GCH):
                    ci = j * GCH + cc
                    nc.tensor.matmul(
                        psLs[s][:, cc * K:(cc + 1) * K],
                        xn_sb[s][:, ci * 128:(ci + 1) * 128],
                        w_sb[:], start=False, stop=(cc == GCH - 1))
            for s in range(BS):
                nc.scalar.activation(
                    out=e0[:, sl, s, :],
                    in_=psLs[s][:].rearrange("p (a b) -> p a b", a=GCH),
                    func=AF.Exp)
            for s in range(BS):
                nc.vector.tensor_reduce(
                    out=ssum[:, s, sl], in_=e0[:, sl, s, :],
                    axis=AX.X, op=ALU.add)
            nc.vector.reciprocal(out=rs[:, :, sl], in_=ssum[:, :, sl])
            for s in range(BS):
                nc.vector.tensor_mul(
                    out=es3[:, sl, s, :], in0=e0[:, sl, s, :],
                    in1=_bc(rs[:, s, sl], GCH, K))

        def emit_burst(j):
            for cc in range(GCH):
                ci = j * GCH + cc
                nc.tensor.matmul(
                    ps_vlad[:, 0:257], es3[:, ci, :, :], rhs_sb[:, ci, :],
                    start=(ci == 0), stop=(ci == NCH - 1))

        # software pipeline: bursts lag so the in-order PE never stalls
        emit_pair(0)
        emit_pair(1)
        emit_pair(2)
        emit_burst(0)
        emit_pair(3)
        emit_burst(1)
        emit_burst(2)
        emit_burst(3)

        # ---- finalize ----
        a_col = ps_vlad[:, 256:257]                 # -sum(a), both samples
        t2 = fin.tile([128, C], f32, tag="t2")
        for s in range(BS):
            vl = ps_vlad[s * K:(s + 1) * K, s * 128:s * 128 + 128]
            # t2 = cent * (-A) + vlad  (the -1 column supplies -A)
            nc.vector.scalar_tensor_tensor(
                out=t2[s * K:(s + 1) * K, :], in0=cent2[s * K:(s + 1) * K, :],
                scalar=a_col[s * K:(s + 1) * K, :], in1=vl,
                op0=ALU.mult, op1=ALU.add)
        sq = fin.tile([128, C], f32, tag="sq")
        rowns = fin.tile([128, 1], f32, tag="rowns")
        nc.scalar.activation(out=sq[:], in_=t2[:], func=AF.Square,
                             accum_out=rowns[:])
        u2 = fin.tile([128, 1], f32, tag="u2")
        nc.vector.reciprocal(out=u2[:], in_=rowns[:])
        rn = fin.tile([128, 1], f32, tag="rn")
        # 1/(8*sqrt(rowns)) = sqrt((1/64)/rowns)
        nc.scalar.activation(out=rn[:], in_=u2[:], func=AF.Sqrt,
                             scale=1.0 / 64.0)
        o_sb = fin.tile([128, C], f32, tag="osb")
        nc.vector.tensor_scalar(out=o_sb[:], in0=t2[:], scalar1=rn[:],
                                scalar2=None, op0=ALU.mult)
        nc.sync.dma_start(
            out=o_h[:, :].rearrange("s (k c) -> (s k) c", c=C), in_=o_sb[:])


_NC = None


def prep_inputs(x, conv_w, conv_b, centroids):
    """Host prep: normalize, cast f16, build per-core input maps."""
    x = np.asarray(x, dtype=np.float32).reshape(B, C, N)
    conv_w = np.asarray(conv_w, dtype=np.float32)
    conv_b = np.asarray(conv_b, dtype=np.float32)
    centroids = np.asarray(centroids, dtype=np.float32)

    ns = np.einsum("bcn,bcn->bn", x, x)
    r = 1.0 / np.sqrt(np.maximum(ns, 1e-24))
    from ml_dtypes import float8_e4m3fn
    xn = (x * r[:, None, :]).astype(float8_e4m3fn)     # [B, C, N]

    from ml_dtypes import float8_e4m3fn as _f8
    wt = np.ascontiguousarray(conv_w.T).astype(_f8)               # [C, K]
    bpack = np.tile(conv_b.astype(np.float16), GCH)[None, :]      # [1, 512]
    bpack = np.ascontiguousarray(bpack)
    cent = np.ascontiguousarray(centroids)

    in_maps = []
    for i in range(NCORES):
        xs = xn[i * BS:(i + 1) * BS]                   # [2, C, N]
        rhs = np.empty((128, NCH, 257), dtype=_f8)
        v = xs.reshape(BS, C, NCH, 128)                # [s, c, ci, p]
        rhs[:, :, 0:128] = v[0].transpose(2, 1, 0)     # -> [p, ci, c]
        rhs[:, :, 128:256] = v[1].transpose(2, 1, 0)
        rhs[:, :, 256:257] = -1.0
        xq = np.ascontiguousarray(
            xs.reshape(BS, C, NG, QW).transpose(0, 2, 1, 3))
        in_maps.append({
            "xn": xq,
            "rhs": rhs,
            "wt": wt,
            "bpack": bpack,
            "cent": cent,
        })
    return in_maps


def kernel(x, conv_w, conv_b, centroids):
    global _NC
    if _NC is None:
        _NC = _build()
    in_maps = prep_inputs(x, conv_w, conv_b, centroids)
    res = run_bass_kernel_spmd(_NC, in_maps, core_ids=list(range(NCORES)))
    return np.concatenate([res.results[i]["out"] for i in range(NCORES)],
                          axis=0)


# revision 16
# speedup vs baseline: 1.0389x; 1.0389x over previous
"""NetVLAD forward on 8 Trainium2 NeuronCores — v2.

Host prep (inside kernel(), CPU): L2-normalize x over channels, cast f16,
and lay out two views: xn (channel-major, mm1 stationary) and rhs_pack
(position-major: [xnT_s0 | xnT_s1 | 1 1] per 128-position chunk, mm2
moving operand). Weights: w^T f16; b tiled 8x; centroids f32.

Device per core (2 samples, C=128, N=4096, K=64):
  mm1 (flipped): per 128-pos chunk, lhsT=xn_chunk -> l0t [n,K] slice in a
    PSUM bank (8 chunks per bank); then one rank-1 matmul per bank
    (ones[1,128] (x) b_tiled[1,512]) accumulates +b[k] onto the whole bank.
  exp: ACT per bank [128,512] PSUM->SBUF f16 (host-normed, b in PSUM).
  ssum = sum_k e0 (DVE reduce), rs = 1/ssum (DVE recip), es3 = e0*rs (DVE).
  mm2: per chunk, lhsT=es3 (both samples packed), rhs=rhs_pack chunk
    [xnT0|xnT1|1 1] -> vlad + A accumulate in PSUM. Bursts are emitted
    lagging two groups behind mm1 so the in-order PE queue never stalls.
  finalize: t2 = vlad - cent*A; row L2 norm; /sqrt(K); DMA out.
"""

import contextlib

import numpy as np

import concourse.bass as bass
import concourse.bacc as bacc
import concourse.tile as tile
from concourse import mybir
from concourse.bass_utils import run_bass_kernel_spmd

f32 = mybir.dt.float32
f16 = mybir.dt.float16
f8 = mybir.dt.float8e4
AF = mybir.ActivationFunctionType
ALU = mybir.AluOpType
AX = mybir.AxisListType

B, C, N, K = 16, 128, 4096, 64
NCORES = 8
BS = B // NCORES          # 2 samples per core
NCH = N // 128            # 32 chunks per sample
GCH = 8                   # chunks per PSUM bank / exp group
NG = NCH // GCH           # 4 bank-groups per sample
QW = GCH * 128            # 1024 columns of xn per group


def _bc(ap, ncols, rep):
    """[P, ncols] tile viewed as [P, ncols, rep], inner dim broadcast."""
    return bass.AP(tensor=ap.tensor, offset=ap.offset,
                   ap=[list(ap.ap[0]), [1, ncols], [0, rep]])


def _bcK(ap2d, rep):
    """[P, n] AP (any stride) viewed as [P, n, rep], inner broadcast."""
    return bass.AP(tensor=ap2d.tensor, offset=ap2d.offset,
                   ap=[list(ap2d.ap[0]), list(ap2d.ap[1]), [0, rep]])


def _build():
    nc = bacc.Bacc("TRN2", target_bir_lowering=False, debug=False,
                   num_devices=NCORES)
    xn_h = nc.dram_tensor("xn", [BS, NG, C, QW], f8, kind="ExternalInput")
    rhs_h = nc.dram_tensor("rhs", [128, NCH, 257], f8, kind="ExternalInput")
    w_h = nc.dram_tensor("wt", [C, K], f8, kind="ExternalInput")
    bp_h = nc.dram_tensor("bpack", [1, GCH * K], f16, kind="ExternalInput")
    c_h = nc.dram_tensor("cent", [K, C], f32, kind="ExternalInput")
    o_h = nc.dram_tensor("out", [BS, K * C], f32, kind="ExternalOutput")

    with tile.TileContext(nc) as tc:
        _emit(nc, tc, xn_h, rhs_h, w_h, bp_h, c_h, o_h)
    nc.compile()
    return nc


def _emit(nc, tc, xn_h, rhs_h, w_h, bp_h, c_h, o_h):
    ctx = contextlib.ExitStack()
    with ctx:
        const = ctx.enter_context(tc.tile_pool(name="const", bufs=1))
        fin = ctx.enter_context(tc.tile_pool(name="fin", bufs=4))
        ps_l = ctx.enter_context(tc.tile_pool(name="ps_l", bufs=6, space="PSUM"))
        ps_v = ctx.enter_context(tc.tile_pool(name="ps_v", bufs=1, space="PSUM"))

        # ---- constants (DMAs posted after xn q0 below) ----
        w_sb = const.tile([C, K], f8, tag="w_sb")
        bp_sb = const.tile([1, GCH * K], f16, tag="bp_sb")
        cent2 = const.tile([128, C], f32, tag="cent2")
        ones_row = const.tile([1, 128], f16, tag="ones_row")
        nc.vector.memset(ones_row[:], 1.0)
        # tiny early exp to pull the ACT Exp table load into the DMA window
        dmy = const.tile([1, 1], f32, tag="dmy")
        nc.vector.memset(dmy[:], 0.0)
        nc.scalar.activation(out=dmy[:], in_=dmy[:], func=AF.Exp)

        # ---- streamed inputs, quarter-granular and interleaved ----
        xn_sb = [const.tile([C, N], f8, tag=f"xn{s}", name=f"xn_sb{s}")
                 for s in range(BS)]
        rhs_sb = const.tile([128, NCH, 257], f8, tag="rhs")
        # first 8 posts = the 8 xn quarters -> unique HWDGE sem lanes;
        # later posts recycle lanes of earlier-landing transfers, which only
        # ever makes a waiter wait on something that finished earlier.
        def xn_span(s, q0, nq, eng):
            cs = slice(q0 * QW, (q0 + nq) * QW)
            src_ap = xn_h[s, q0:q0 + nq, :, :]        # [nq, C, QW]
            src_cqw = bass.AP(tensor=src_ap.tensor, offset=src_ap.offset,
                              ap=[list(src_ap.ap[1]), list(src_ap.ap[0]),
                                  list(src_ap.ap[2])])
            eng.dma_start(
                out=xn_sb[s][:, cs].rearrange("c (q w) -> c q w", q=nq),
                in_=src_cqw)
        # tiny first posts unblock pair 0 ASAP; one big post carries the rest
        xn_span(0, 0, 1, nc.sync)
        xn_span(1, 0, 1, nc.scalar)
        nc.gpsimd.dma_start(out=bp_sb[:], in_=bp_h[:, :])
        xn_span(0, 1, 3, nc.sync)
        xn_span(1, 1, 3, nc.scalar)
        nc.gpsimd.dma_start(out=w_sb[:], in_=w_h[:, :])
        nc.gpsimd.dma_start(out=rhs_sb[:], in_=rhs_h[:, :, :])
        nc.sync.dma_start(out=cent2[0:K, :], in_=c_h[:, :])
        nc.scalar.dma_start(out=cent2[K:128, :], in_=c_h[:, :])

        # pipeline tiles [128, ci, s, K] (chunk-major: mm2 lhsT slice is
        # contiguous) and per-position softmax scalars
        e0 = const.tile([128, NCH, BS, K], f16, tag="e0")
        es3 = const.tile([128, NCH, BS, K], f8, tag="es3")
        ssum = const.tile([128, BS, NCH], f32, tag="ssum")
        rs = const.tile([128, BS, NCH], f32, tag="rs")

        ps_vlad = ps_v.tile([128, 257], f32, tag="vlad")

        preseed = {}
        for j in range(3):
            for s in range(BS):
                psL = ps_l.tile([128, GCH * K], f32, tag="l0t",
                                name=f"psL_{j}_{s}")
                nc.tensor.matmul(psL[:], ones_row[0:1, :], bp_sb[0:1, :],
                                 start=True, stop=False)
                preseed[(j, s)] = psL

        def emit_pair(j):
            sl = slice(j * GCH, (j + 1) * GCH)
            psLs = []
            for s in range(BS):
                if (j, s) in preseed:
                    psLs.append(preseed[(j, s)])
                    continue
                psL = ps_l.tile([128, GCH * K], f32, tag="l0t",
                                name=f"psL_{j}_{s}")
                nc.tensor.matmul(psL[:], ones_row[0:1, :], bp_sb[0:1, :],
                                 start=True, stop=False)
                psLs.append(psL)
            for s in range(BS):
                for cc in range(GCH):
                    ci = j * GCH + cc
                    nc.tensor.matmul(
                        psLs[s][:, cc * K:(cc + 1) * K],
                        xn_sb[s][:, ci * 128:(ci + 1) * 128],
                        w_sb[:], start=False, stop=(cc == GCH - 1))
            for s in range(BS):
                nc.scalar.activation(
                    out=e0[:, sl, s, :],
                    in_=psLs[s][:].rearrange("p (a b) -> p a b", a=GCH),
                    func=AF.Exp)
            for s in range(BS):
                nc.vector.tensor_reduce(
                    out=ssum[:, s, sl], in_=e0[:, sl, s, :],
                    axis=AX.X, op=ALU.add)
            nc.vector.reciprocal(out=rs[:, :, sl], in_=ssum[:, :, sl])
            for s in range(BS):
                nc.vector.tensor_mul(
                    out=es3[:, sl, s, :], in0=e0[:, sl, s, :],
                    in1=_bc(rs[:, s, sl], GCH, K))

        def emit_burst(j):
            for cc in range(GCH):
                ci = j * GCH + cc
                nc.tensor.matmul(
                    ps_vlad[:, 0:257], es3[:, ci, :, :], rhs_sb[:, ci, :],
                    start=(ci == 0), stop=(ci == NCH - 1))

        # software pipeline: bursts lag so the in-order PE never stalls
        emit_pair(0)
        emit_pair(1)
        emit_pair(2)
        emit_burst(0)
        emit_pair(3)
        emit_burst(1)
        emit_burst(2)
        emit_burst(3)

        # ---- finalize ----
        a_col = ps_vlad[:, 256:257]                 # -sum(a), both samples
        t2 = fin.tile([128, C], f32, tag="t2")
        for s in range(BS):
            vl = ps_vlad[s * K:(s + 1) * K, s * 128:s * 128 + 128]
            # t2 = cent * (-A) + vlad  (the -1 column supplies -A)
            nc.vector.scalar_tensor_tensor(
                out=t2[s * K:(s + 1) * K, :], in0=cent2[s * K:(s + 1) * K, :],
                scalar=a_col[s * K:(s + 1) * K, :], in1=vl,
                op0=ALU.mult, op1=ALU.add)
        sq = fin.tile([128, C], f32, tag="sq")
        nc.vector.tensor_mul(out=sq[:], in0=t2[:], in1=t2[:])
        rowns = fin.tile([128, 1], f32, tag="rowns")
        nc.vector.tensor_reduce(out=rowns[:], in_=sq[:], axis=AX.X,
                                op=ALU.add)
        u2 = fin.tile([128, 1], f32, tag="u2")
        nc.vector.reciprocal(out=u2[:], in_=rowns[:])
        rn = fin.tile([128, 1], f32, tag="rn")
        # 1/(8*sqrt(rowns)) = sqrt((1/64)/rowns)
        nc.scalar.activation(out=rn[:], in_=u2[:], func=AF.Sqrt,
                             scale=1.0 / 64.0)
        o_sb = fin.tile([128, C], f32, tag="osb")
        nc.vector.tensor_scalar(out=o_sb[:], in0=t2[:], scalar1=rn[:],
                                scalar2=None, op0=ALU.mult)
        nc.sync.dma_start(
            out=o_h[:, :].rearrange("s (k c) -> (s k) c", c=C), in_=o_sb[:])


_NC = None


def prep_inputs(x, conv_w, conv_b, centroids):
    """Host prep: normalize, cast f16, build per-core input maps."""
    x = np.asarray(x, dtype=np.float32).reshape(B, C, N)
    conv_w = np.asarray(conv_w, dtype=np.float32)
    conv_b = np.asarray(conv_b, dtype=np.float32)
    centroids = np.asarray(centroids, dtype=np.float32)

    ns = np.einsum("bcn,bcn->bn", x, x)
    r = 1.0 / np.sqrt(np.maximum(ns, 1e-24))
    from ml_dtypes import float8_e4m3fn
    xn = (x * r[:, None, :]).astype(float8_e4m3fn)     # [B, C, N]

    from ml_dtypes import float8_e4m3fn as _f8
    wt = np.ascontiguousarray(conv_w.T).astype(_f8)               # [C, K]
    bpack = np.tile(conv_b.astype(np.float16), GCH)[None, :]      # [1, 512]
    bpack = np.ascontiguousarray(bpack)
    cent = np.ascontiguousarray(centroids)

    in_maps = []
    for i in range(NCORES):
        xs = xn[i * BS:(i + 1) * BS]                   # [2, C, N]
        rhs = np.empty((128, NCH, 257), dtype=_f8)
        v = xs.reshape(BS, C, NCH, 128)                # [s, c, ci, p]
        rhs[:, :, 0:128] = v[0].transpose(2, 1, 0)     # -> [p, ci, c]
        rhs[:, :, 128:256] = v[1].transpose(2, 1, 0)
        rhs[:, :, 256:257] = -1.0
        xq = np.ascontiguousarray(
            xs.reshape(BS, C, NG, QW).transpose(0, 2, 1, 3))
        in_maps.append({
            "xn": xq,
            "rhs": rhs,
            "wt": wt,
            "bpack": bpack,
            "cent": cent,
        })
    return in_maps


def kernel(x, conv_w, conv_b, centroids):
    global _NC
    if _NC is None:
        _NC = _build()
    in_maps = prep_inputs(x, conv_w, conv_b, centroids)
    res = run_bass_kernel_spmd(_NC, in_maps, core_ids=list(range(NCORES)))
    return np.concatenate([res.results[i]["out"] for i in range(NCORES)],
                          axis=0)
